# revision 57
# baseline (speedup 1.0000x reference)
"""CAM (channel attention) module kernel for Trainium2, 8-core data-parallel.

Computes, per batch b (one batch per NeuronCore):
    q = x[b].reshape(C, N)                  # C=512, N=4096
    E = q @ q.T                             # [C, C], symmetric
    att = softmax(rowmax(E) - E, axis=-1)   # == exp(rowmin(E)-E)/rowsum
    out = gamma * (att @ q) + x[b]

v2 design (fp32 matmul on trn2 is 2-pass LOW_HIGH emulation, ~5.5x slower
than bf16 -> do all matmuls in bf16, keep the +x and final scaling in fp32):
  - q loaded fp32 [128, 4, 4096]; cast to bf16 on DVE/ACT in 16 chunks.
  - qT built with DMA xbar transposes (bf16) into [128, 4(c), 32(k), 128],
    no PE or DVE time spent on transposition.
  - per channel-tile i (fused pipeline, overlaps across i):
      energy: 32 matmuls (lhsT=qT[:,i,k,:], rhs=qT[:,:,k,:]) accum in PSUM
      softmax: rowmin (DVE) -> exp(mn-E) on ACT writing bf16 att +
               fused row-sum (accum_out); rg = gamma/s kept per-partition
      attT: one DMA xbar transpose of att[:,i,:] -> [128, 4(j), 128]
      out: per 512-col chunk: 4 bf16 matmuls + one DVE
           scalar_tensor_tensor: out = (psum * rg) + x  (exact fp32 x-add)
      out DMA per chunk pair.
  - att is left unnormalized; gamma/s scaling rides the final DVE op, so
    gamma=0 gives out == x exactly.
"""

import sys

import numpy as np

for _p in ("/opt/trn_rl_repo",):
    if _p not in sys.path:
        sys.path.insert(0, _p)

B, C, H, W = 8, 512, 64, 64
N = H * W  # 4096
P = 128
CT = C // P  # 4 channel tiles
KT = N // P  # 32 spatial tiles
FD = 512  # matmul free-dim / PSUM bank width (fp32)
NCH = N // FD  # 8 output column chunks
LCH = 4  # input load chunks per c-tile
LW = N // LCH  # 1024

_CACHE = {}


def _build_bass():
    import concourse.mybir as mybir
    import concourse.tile as tile
    from concourse import bacc
    from concourse.masks import make_identity

    fp32 = mybir.dt.float32
    bf16 = mybir.dt.bfloat16
    AX = mybir.AxisListType.X
    ALU = mybir.AluOpType
    ACT_EXP = mybir.ActivationFunctionType.Exp

    nc = bacc.Bacc(None, target_bir_lowering=False, debug=False)
    x_d = nc.dram_tensor("x", [C, N], fp32, kind="ExternalInput")
    g_d = nc.dram_tensor("gamma", [1], fp32, kind="ExternalInput")
    o_d = nc.dram_tensor("out", [C, N], fp32, kind="ExternalOutput")

    with tile.TileContext(nc) as tc:
        with (
            tc.tile_pool(name="persist", bufs=1) as persist,
            tc.tile_pool(name="stats", bufs=4) as stats,
            tc.tile_pool(name="outp", bufs=4) as outp,
            tc.tile_pool(name="epsum", bufs=4, space="PSUM") as epsum,
            tc.tile_pool(name="opsum", bufs=2, space="PSUM") as opsum,
        ):
            gam = persist.tile([P, 1], fp32)
            ident = persist.tile([P, P], bf16)
            make_identity(nc, ident)
            ident32 = persist.tile([P, P], fp32)
            make_identity(nc, ident32)
            q = persist.tile([P, CT, N], fp32)
            q_bf = persist.tile([P, CT, N], bf16)
            # k-major qT: qT[p, k, c, v] = q[c*128+v, k*128+p]; energy rhs for
            # chunk k is the contiguous [128, 512] slab qT[:, k, :, :]
            qT = persist.tile([P, KT, CT, P], bf16)
            # per-(c,h) xbar transpose staging (separate tiles so the 8
            # transposes don't serialize on a shared-tile WAW dep)
            qTs = [
                persist.tile([P, KT // 4, P], bf16, name=f"qTs{j}", tag=f"qTs{j}")
                for j in range(4 * CT)
            ]
            att = persist.tile([P, CT, C], bf16)
            attT = persist.tile([P, CT, CT, P], bf16)

            # load fp32 in 1MB chunks, cast to bf16 on DVE, xbar-transpose.
            # Each issuing engine owns one DMA ring (FIFO), so: loads split
            # over gpsimd(SWDGE)+scalar(ACT-HWDGE), transposes split over
            # sync(SP-HWDGE)+scalar, stores on gpsimd. A transpose waiting on
            # its cast must never sit ahead of a load in the same ring.
            NCHK = 4 * CT  # 16 load/cast/transpose chunks of 1024 cols
            # (h-major order so early chunks cover every c)
            CW = N // (NCHK // CT)  # 1024
            chunks = [(h, c) for h in range(NCHK // CT) for c in range(CT)]
            nc.gpsimd.dma_start(out=gam, in_=g_d[:].to_broadcast((P, 1)))

            def load(idx):
                h, c = chunks[idx]
                sl = slice(h * CW, (h + 1) * CW)
                ring = nc.sync if c % 2 == 0 else nc.scalar
                ring.dma_start(out=q[:, c, sl], in_=x_d[c * P : (c + 1) * P, sl])

            def cast(idx):
                h, c = chunks[idx]
                sl = slice(h * CW, (h + 1) * CW)
                # split casts DVE/GpSimd: the serial DVE cast chain gates each
                # h-group's PE transposes, and GpSimd is compute-idle here;
                # fewer DVE ops also moves the energy gates (encoded as
                # DVE-op-count positions) earlier.
                eng = nc.vector if c % 2 == 0 else nc.gpsimd
                eng.tensor_copy(out=q_bf[:, c, sl], in_=q[:, c, sl])

            def transpose(idx):
                h, c = chunks[idx]
                sl = slice(h * CW, (h + 1) * CW)
                ring = nc.sync if c % 2 == 0 else nc.scalar
                ring.dma_start_transpose(out=qTs[idx], in_=q_bf[:, c, sl])

            def gather(idx):
                h, c = chunks[idx]
                ksl = slice(h * (CW // P), (h + 1) * (CW // P))
                nc.vector.tensor_copy(out=qT[:, ksl, c, :], in_=qTs[idx])

            # Interleaved pipeline on the two HWDGE rings (gpsimd/SWDGE is too
            # slow — its in-flight DMAs hold shared lane sems that stall
            # everyone). Each transpose rides 2 load-slots behind its own load
            # (cast just done when the ring reaches it); gathers sit early in
            # DVE order (Tile encodes cross-engine deps as DVE-op-count).
            # h0 (chunks 0-3) is transposed on the then-idle PE instead of the
            # xbar: its 4 chunks gate the first energy matmuls, and early xbar
            # transposes crawl while the load stream owns the SBUF ports.
            def pe_transpose(idx):
                h, c = chunks[idx]
                kb = CW // P  # k-blocks per chunk
                tp = opsum.tile([P, 4 * FD], bf16, name="tpb", tag="ops")
                for kk in range(kb):
                    nc.tensor.transpose(
                        tp[:, kk * P : (kk + 1) * P],
                        q_bf[:, c, (h * kb + kk) * P : (h * kb + kk + 1) * P],
                        ident,
                    )
                nc.vector.tensor_copy(
                    out=qT[:, h * kb : (h + 1) * kb, c, :],
                    in_=tp[:, 0:CW].rearrange("p (k v) -> p k v", v=P),
                )

            # ---- energy accumulators ----
            Es = [
                epsum.tile([P, C], fp32, name=f"E{i}", tag=f"E{i}", bufs=1)
                for i in range(CT)
            ]

            # E is symmetric: compute only column blocks j >= i; the j < i
            # blocks are mirrored from E[j] after accumulation completes.
            def energy(k0, k1, stop=False):
                for k in range(k0, k1):
                    for i in range(CT):
                        nc.tensor.matmul(
                            Es[i][:, i * P :],
                            lhsT=qT[:, k, i, :],
                            rhs=qT[:, k, i:, :],
                            start=(k == 0),
                            stop=(stop and k == KT - 1),
                        )

            for idx in range(NCHK):
                load(idx)
                for g in range(3):  # h0..h2 on PE, energy group right after
                    if idx == 4 * g + 3:
                        for j in range(4 * g, 4 * g + 4):
                            cast(j)
                        for jj in range(4 * g, 4 * g + 4):
                            pe_transpose(jj)
                        energy(8 * g, 8 * g + 8)
                if idx >= 12:
                    cast(idx)
            for idx in range(NCHK - 4, NCHK):
                transpose(idx)
            for idx in range(NCHK - 4, NCHK):
                gather(idx)

            # switch to i-outer for the tail so E[i] completes (and
            # softmax(i) starts) staggered in i; then mirror E[i, j<i] from
            # the finished E[j] row-blocks (copy out to SBUF, PE-transpose
            # back into the bank).
            for i in range(CT):
                for k in range(24, KT):
                    nc.tensor.matmul(
                        Es[i][:, i * P :],
                        lhsT=qT[:, k, i, :],
                        rhs=qT[:, k, i:, :],
                        start=False,
                        stop=(k == KT - 1),
                    )
                for j in range(i):
                    etmp = stats.tile([P, P], fp32, name="etmp", tag="etmp")
                    nc.vector.tensor_copy(
                        out=etmp, in_=Es[j][:, i * P : (i + 1) * P]
                    )
                    nc.tensor.transpose(
                        Es[i][:, j * P : (j + 1) * P], etmp, ident32
                    )

            for i in range(CT):
                E = Es[i]
                # ---- softmax (unnormalized): att = exp(mn - E), s = rowsum ----
                mn = stats.tile([P, 1], fp32)
                nc.vector.tensor_reduce(out=mn, in_=E, axis=AX, op=ALU.min)
                s = stats.tile([P, 1], fp32)
                nc.scalar.activation(
                    out=att[:, i, :],
                    in_=E,
                    func=ACT_EXP,
                    bias=mn,
                    scale=-1.0,
                    accum_out=s,
                )
                rg = stats.tile([P, 1], fp32)
                nc.vector.reciprocal(out=rg, in_=s)
                nc.vector.tensor_mul(rg, rg, gam)

                # ---- attT slab i via xbar transpose ----
                tr = nc.sync if i % 2 == 0 else nc.scalar
                tr.dma_start_transpose(out=attT[:, i, :, :], in_=att[:, i, :])

                # ---- out row-block i ----
                ot = outp.tile([P, N], fp32, name="ot", tag="ot", bufs=2)
                for nh in range(4):  # 1024-wide chunks, 2 PSUM banks each
                    sl = slice(nh * 2 * FD, (nh + 1) * 2 * FD)
                    ops = opsum.tile([P, 2 * FD], fp32, name="ops", tag="ops")
                    for half in range(2):
                        hsl = slice((nh * 2 + half) * FD, (nh * 2 + half + 1) * FD)
                        for j in range(CT):
                            nc.tensor.matmul(
                                ops[:, half * FD : (half + 1) * FD],
                                lhsT=attT[:, i, j, :],
                                rhs=q_bf[:, j, hsl],
                                start=(j == 0),
                                stop=(j == CT - 1),
                            )
                    # out = (psum * gamma/s) + x, exact fp32 add of x
                    nc.vector.scalar_tensor_tensor(
                        out=ot[:, sl],
                        in0=ops,
                        scalar=rg,
                        in1=q[:, i, sl],
                        op0=ALU.mult,
                        op1=ALU.add,
                    )
                    st = [nc.sync, nc.scalar, nc.gpsimd][(i * 4 + nh) % 3]
                    st.dma_start(out=o_d[i * P : (i + 1) * P, sl], in_=ot[:, sl])

    nc.compile()
    return nc


def _get_nc():
    if "nc" not in _CACHE:
        _CACHE["nc"] = _build_bass()
    return _CACHE["nc"]


def run(x, gamma, **run_kwargs):
    """Run on 8 cores; returns (results_list, BassKernelResults)."""
    from concourse.bass_utils import run_bass_kernel_spmd

    nc = _get_nc()
    x = np.ascontiguousarray(x, dtype=np.float32)
    gamma = np.ascontiguousarray(gamma, dtype=np.float32)
    in_maps = [
        {"x": np.ascontiguousarray(x[b].reshape(C, N)), "gamma": gamma}
        for b in range(B)
    ]
    res = run_bass_kernel_spmd(nc, in_maps, core_ids=list(range(B)), **run_kwargs)
    out = np.stack([r["out"] for r in res.results]).reshape(B, C, H, W)
    return out, res


def kernel(x, gamma):
    out, _ = run(x, gamma)
    return out.astype(np.float32)


# revision 59
# speedup vs baseline: 1.1189x; 1.1189x over previous
"""CAM (channel attention) module kernel for Trainium2, 8-core data-parallel.

Computes, per batch b (one batch per NeuronCore):
    q = x[b].reshape(C, N)                  # C=512, N=4096
    E = q @ q.T                             # [C, C], symmetric
    att = softmax(rowmax(E) - E, axis=-1)   # == exp(rowmin(E)-E)/rowsum
    out = gamma * (att @ q) + x[b]

v2 design (fp32 matmul on trn2 is 2-pass LOW_HIGH emulation, ~5.5x slower
than bf16 -> do all matmuls in bf16, keep the +x and final scaling in fp32):
  - q loaded fp32 [128, 4, 4096]; cast to bf16 on DVE/ACT in 16 chunks.
  - qT built with DMA xbar transposes (bf16) into [128, 4(c), 32(k), 128],
    no PE or DVE time spent on transposition.
  - per channel-tile i (fused pipeline, overlaps across i):
      energy: 32 matmuls (lhsT=qT[:,i,k,:], rhs=qT[:,:,k,:]) accum in PSUM
      softmax: rowmin (DVE) -> exp(mn-E) on ACT writing bf16 att +
               fused row-sum (accum_out); rg = gamma/s kept per-partition
      attT: one DMA xbar transpose of att[:,i,:] -> [128, 4(j), 128]
      out: per 512-col chunk: 4 bf16 matmuls + one DVE
           scalar_tensor_tensor: out = (psum * rg) + x  (exact fp32 x-add)
      out DMA per chunk pair.
  - att is left unnormalized; gamma/s scaling rides the final DVE op, so
    gamma=0 gives out == x exactly.
"""

import sys

import numpy as np

for _p in ("/opt/trn_rl_repo",):
    if _p not in sys.path:
        sys.path.insert(0, _p)

B, C, H, W = 8, 512, 64, 64
N = H * W  # 4096
P = 128
CT = C // P  # 4 channel tiles
KT = N // P  # 32 spatial tiles
FD = 512  # matmul free-dim / PSUM bank width (fp32)
NCH = N // FD  # 8 output column chunks
LCH = 4  # input load chunks per c-tile
LW = N // LCH  # 1024

_CACHE = {}


def _build_bass():
    import concourse.mybir as mybir
    import concourse.tile as tile
    from concourse import bacc
    from concourse.masks import make_identity

    fp32 = mybir.dt.float32
    bf16 = mybir.dt.bfloat16
    AX = mybir.AxisListType.X
    ALU = mybir.AluOpType
    ACT_EXP = mybir.ActivationFunctionType.Exp

    nc = bacc.Bacc(None, target_bir_lowering=False, debug=False)
    x_d = nc.dram_tensor("x", [C, N], fp32, kind="ExternalInput")
    g_d = nc.dram_tensor("gamma", [1], fp32, kind="ExternalInput")
    o_d = nc.dram_tensor("out", [C, N], fp32, kind="ExternalOutput")

    with tile.TileContext(nc) as tc:
        with (
            tc.tile_pool(name="persist", bufs=1) as persist,
            tc.tile_pool(name="stats", bufs=4) as stats,
            tc.tile_pool(name="outp", bufs=4) as outp,
            tc.tile_pool(name="epsum", bufs=4, space="PSUM") as epsum,
            tc.tile_pool(name="opsum", bufs=2, space="PSUM") as opsum,
        ):
            gam = persist.tile([P, 1], fp32)
            ident = persist.tile([P, P], bf16)
            make_identity(nc, ident)
            ident32 = persist.tile([P, P], fp32)
            make_identity(nc, ident32)
            q = persist.tile([P, CT, N], fp32)
            q_bf = persist.tile([P, CT, N], bf16)
            # k-major qT: qT[p, k, c, v] = q[c*128+v, k*128+p]; energy rhs for
            # chunk k is the contiguous [128, 512] slab qT[:, k, :, :]
            qT = persist.tile([P, KT, CT, P], bf16)
            # per-(c,h) xbar transpose staging (separate tiles so the 8
            # transposes don't serialize on a shared-tile WAW dep)
            qTs = [
                persist.tile([P, KT // 4, P], bf16, name=f"qTs{j}", tag=f"qTs{j}")
                for j in range(4 * CT)
            ]
            att = persist.tile([P, CT, C], bf16)
            attT = persist.tile([P, CT, CT, P], bf16)

            # load fp32 in 1MB chunks, cast to bf16 on DVE, xbar-transpose.
            # Each issuing engine owns one DMA ring (FIFO), so: loads split
            # over gpsimd(SWDGE)+scalar(ACT-HWDGE), transposes split over
            # sync(SP-HWDGE)+scalar, stores on gpsimd. A transpose waiting on
            # its cast must never sit ahead of a load in the same ring.
            NCHK = 4 * CT  # 16 load/cast/transpose chunks of 1024 cols
            # (h-major order so early chunks cover every c)
            CW = N // (NCHK // CT)  # 1024
            chunks = [(h, c) for h in range(NCHK // CT) for c in range(CT)]
            nc.gpsimd.dma_start(out=gam, in_=g_d[:].to_broadcast((P, 1)))

            def load(idx):
                h, c = chunks[idx]
                sl = slice(h * CW, (h + 1) * CW)
                ring = nc.sync if c % 2 == 0 else nc.scalar
                ring.dma_start(out=q[:, c, sl], in_=x_d[c * P : (c + 1) * P, sl])

            def cast(idx):
                h, c = chunks[idx]
                sl = slice(h * CW, (h + 1) * CW)
                nc.vector.tensor_copy(out=q_bf[:, c, sl], in_=q[:, c, sl])

            def transpose(idx):
                h, c = chunks[idx]
                sl = slice(h * CW, (h + 1) * CW)
                ring = nc.sync if c % 2 == 0 else nc.scalar
                ring.dma_start_transpose(out=qTs[idx], in_=q_bf[:, c, sl])

            def gather(idx):
                h, c = chunks[idx]
                ksl = slice(h * (CW // P), (h + 1) * (CW // P))
                nc.vector.tensor_copy(out=qT[:, ksl, c, :], in_=qTs[idx])

            # Interleaved pipeline on the two HWDGE rings (gpsimd/SWDGE is too
            # slow — its in-flight DMAs hold shared lane sems that stall
            # everyone). Each transpose rides 2 load-slots behind its own load
            # (cast just done when the ring reaches it); gathers sit early in
            # DVE order (Tile encodes cross-engine deps as DVE-op-count).
            # h0 (chunks 0-3) is transposed on the then-idle PE instead of the
            # xbar: its 4 chunks gate the first energy matmuls, and early xbar
            # transposes crawl while the load stream owns the SBUF ports.
            def pe_transpose(idx):
                h, c = chunks[idx]
                kb = CW // P  # k-blocks per chunk
                tp = opsum.tile([P, 4 * FD], bf16, name="tpb", tag="ops")
                for kk in range(kb):
                    nc.tensor.transpose(
                        tp[:, kk * P : (kk + 1) * P],
                        q_bf[:, c, (h * kb + kk) * P : (h * kb + kk + 1) * P],
                        ident,
                    )
                nc.vector.tensor_copy(
                    out=qT[:, h * kb : (h + 1) * kb, c, :],
                    in_=tp[:, 0:CW].rearrange("p (k v) -> p k v", v=P),
                )

            # ---- energy accumulators ----
            Es = [
                epsum.tile([P, C], fp32, name=f"E{i}", tag=f"E{i}", bufs=1)
                for i in range(CT)
            ]

            # E is symmetric: compute only column blocks j >= i; the j < i
            # blocks are mirrored from E[j] after accumulation completes.
            def energy(k0, k1, stop=False):
                for k in range(k0, k1):
                    for i in range(CT):
                        nc.tensor.matmul(
                            Es[i][:, i * P :],
                            lhsT=qT[:, k, i, :],
                            rhs=qT[:, k, i:, :],
                            start=(k == 0),
                            stop=(stop and k == KT - 1),
                        )

            for idx in range(NCHK):
                load(idx)
                for g in range(3):  # h0..h2 on PE, energy group right after
                    if idx == 4 * g + 3:
                        for j in range(4 * g, 4 * g + 4):
                            cast(j)
                        for jj in range(4 * g, 4 * g + 4):
                            pe_transpose(jj)
                        energy(8 * g, 8 * g + 8)
                if idx >= 12:
                    cast(idx)
            for idx in range(NCHK - 4, NCHK):
                transpose(idx)
            for idx in range(NCHK - 4, NCHK):
                gather(idx)
            energy(24, 28)

            # switch to i-outer for the tail so E[i] completes (and
            # softmax(i) starts) staggered in i; then mirror E[i, j<i] from
            # the finished E[j] row-blocks (copy out to SBUF, PE-transpose
            # back into the bank).
            for i in range(CT):
                for k in range(28, KT):
                    nc.tensor.matmul(
                        Es[i][:, i * P :],
                        lhsT=qT[:, k, i, :],
                        rhs=qT[:, k, i:, :],
                        start=False,
                        stop=(k == KT - 1),
                    )
                for j in range(i):
                    etmp = stats.tile([P, P], fp32, name="etmp", tag="etmp")
                    nc.vector.tensor_copy(
                        out=etmp, in_=Es[j][:, i * P : (i + 1) * P]
                    )
                    nc.tensor.transpose(
                        Es[i][:, j * P : (j + 1) * P], etmp, ident32
                    )

            for i in range(CT):
                E = Es[i]
                # ---- softmax (unnormalized): att = exp(mn - E), s = rowsum ----
                mn = stats.tile([P, 1], fp32)
                nc.vector.tensor_reduce(out=mn, in_=E, axis=AX, op=ALU.min)
                s = stats.tile([P, 1], fp32)
                nc.scalar.activation(
                    out=att[:, i, :],
                    in_=E,
                    func=ACT_EXP,
                    bias=mn,
                    scale=-1.0,
                    accum_out=s,
                )
                rg = stats.tile([P, 1], fp32)
                nc.vector.reciprocal(out=rg, in_=s)
                nc.vector.tensor_mul(rg, rg, gam)

                # ---- attT slab i via xbar transpose ----
                tr = nc.sync if i % 2 == 0 else nc.scalar
                tr.dma_start_transpose(out=attT[:, i, :, :], in_=att[:, i, :])

                # ---- out row-block i ----
                ot = outp.tile([P, N], fp32, name="ot", tag="ot", bufs=2)
                for nh in range(4):  # 1024-wide chunks, 2 PSUM banks each
                    sl = slice(nh * 2 * FD, (nh + 1) * 2 * FD)
                    ops = opsum.tile([P, 2 * FD], fp32, name="ops", tag="ops")
                    for half in range(2):
                        hsl = slice((nh * 2 + half) * FD, (nh * 2 + half + 1) * FD)
                        for j in range(CT):
                            nc.tensor.matmul(
                                ops[:, half * FD : (half + 1) * FD],
                                lhsT=attT[:, i, j, :],
                                rhs=q_bf[:, j, hsl],
                                start=(j == 0),
                                stop=(j == CT - 1),
                            )
                    # out = (psum * gamma/s) + x, exact fp32 add of x
                    nc.vector.scalar_tensor_tensor(
                        out=ot[:, sl],
                        in0=ops,
                        scalar=rg,
                        in1=q[:, i, sl],
                        op0=ALU.mult,
                        op1=ALU.add,
                    )
                    st = [nc.sync, nc.scalar, nc.gpsimd][(i * 4 + nh) % 3]
                    st.dma_start(out=o_d[i * P : (i + 1) * P, sl], in_=ot[:, sl])

    nc.compile()
    return nc


def _get_nc():
    if "nc" not in _CACHE:
        _CACHE["nc"] = _build_bass()
    return _CACHE["nc"]


def run(x, gamma, **run_kwargs):
    """Run on 8 cores; returns (results_list, BassKernelResults)."""
    from concourse.bass_utils import run_bass_kernel_spmd

    nc = _get_nc()
    x = np.ascontiguousarray(x, dtype=np.float32)
    gamma = np.ascontiguousarray(gamma, dtype=np.float32)
    in_maps = [
        {"x": np.ascontiguousarray(x[b].reshape(C, N)), "gamma": gamma}
        for b in range(B)
    ]
    res = run_bass_kernel_spmd(nc, in_maps, core_ids=list(range(B)), **run_kwargs)
    out = np.stack([r["out"] for r in res.results]).reshape(B, C, H, W)
    return out, res


def kernel(x, gamma):
    out, _ = run(x, gamma)
    return out.astype(np.float32)


# revision 60
# speedup vs baseline: 1.1427x; 1.0213x over previous
"""CAM (channel attention) module kernel for Trainium2, 8-core data-parallel.

Computes, per batch b (one batch per NeuronCore):
    q = x[b].reshape(C, N)                  # C=512, N=4096
    E = q @ q.T                             # [C, C], symmetric
    att = softmax(rowmax(E) - E, axis=-1)   # == exp(rowmin(E)-E)/rowsum
    out = gamma * (att @ q) + x[b]

v2 design (fp32 matmul on trn2 is 2-pass LOW_HIGH emulation, ~5.5x slower
than bf16 -> do all matmuls in bf16, keep the +x and final scaling in fp32):
  - q loaded fp32 [128, 4, 4096]; cast to bf16 on DVE/ACT in 16 chunks.
  - qT built with DMA xbar transposes (bf16) into [128, 4(c), 32(k), 128],
    no PE or DVE time spent on transposition.
  - per channel-tile i (fused pipeline, overlaps across i):
      energy: 32 matmuls (lhsT=qT[:,i,k,:], rhs=qT[:,:,k,:]) accum in PSUM
      softmax: rowmin (DVE) -> exp(mn-E) on ACT writing bf16 att +
               fused row-sum (accum_out); rg = gamma/s kept per-partition
      attT: one DMA xbar transpose of att[:,i,:] -> [128, 4(j), 128]
      out: per 512-col chunk: 4 bf16 matmuls + one DVE
           scalar_tensor_tensor: out = (psum * rg) + x  (exact fp32 x-add)
      out DMA per chunk pair.
  - att is left unnormalized; gamma/s scaling rides the final DVE op, so
    gamma=0 gives out == x exactly.
"""

import sys

import numpy as np

for _p in ("/opt/trn_rl_repo",):
    if _p not in sys.path:
        sys.path.insert(0, _p)

B, C, H, W = 8, 512, 64, 64
N = H * W  # 4096
P = 128
CT = C // P  # 4 channel tiles
KT = N // P  # 32 spatial tiles
FD = 512  # matmul free-dim / PSUM bank width (fp32)
NCH = N // FD  # 8 output column chunks
LCH = 4  # input load chunks per c-tile
LW = N // LCH  # 1024

_CACHE = {}


def _build_bass():
    import concourse.mybir as mybir
    import concourse.tile as tile
    from concourse import bacc
    from concourse.masks import make_identity

    fp32 = mybir.dt.float32
    bf16 = mybir.dt.bfloat16
    AX = mybir.AxisListType.X
    ALU = mybir.AluOpType
    ACT_EXP = mybir.ActivationFunctionType.Exp

    nc = bacc.Bacc(None, target_bir_lowering=False, debug=False)
    x_d = nc.dram_tensor("x", [C, N], fp32, kind="ExternalInput")
    g_d = nc.dram_tensor("gamma", [1], fp32, kind="ExternalInput")
    o_d = nc.dram_tensor("out", [C, N], fp32, kind="ExternalOutput")

    with tile.TileContext(nc) as tc:
        with (
            tc.tile_pool(name="persist", bufs=1) as persist,
            tc.tile_pool(name="stats", bufs=4) as stats,
            tc.tile_pool(name="outp", bufs=4) as outp,
            tc.tile_pool(name="epsum", bufs=4, space="PSUM") as epsum,
            tc.tile_pool(name="opsum", bufs=2, space="PSUM") as opsum,
        ):
            gam = persist.tile([P, 1], fp32)
            ident = persist.tile([P, P], bf16)
            make_identity(nc, ident)
            ident32 = persist.tile([P, P], fp32)
            make_identity(nc, ident32)
            q = persist.tile([P, CT, N], fp32)
            q_bf = persist.tile([P, CT, N], bf16)
            # k-major qT: qT[p, k, c, v] = q[c*128+v, k*128+p]; energy rhs for
            # chunk k is the contiguous [128, 512] slab qT[:, k, :, :]
            qT = persist.tile([P, KT, CT, P], bf16)
            # per-(c,h) xbar transpose staging (separate tiles so the 8
            # transposes don't serialize on a shared-tile WAW dep)
            qTs = [
                persist.tile([P, KT // 4, P], bf16, name=f"qTs{j}", tag=f"qTs{j}")
                for j in range(4 * CT)
            ]
            att = persist.tile([P, CT, C], bf16)
            attT = persist.tile([P, CT, CT, P], bf16)

            # load fp32 in 1MB chunks, cast to bf16 on DVE, xbar-transpose.
            # Each issuing engine owns one DMA ring (FIFO), so: loads split
            # over gpsimd(SWDGE)+scalar(ACT-HWDGE), transposes split over
            # sync(SP-HWDGE)+scalar, stores on gpsimd. A transpose waiting on
            # its cast must never sit ahead of a load in the same ring.
            NCHK = 4 * CT  # 16 load/cast/transpose chunks of 1024 cols
            # (h-major order so early chunks cover every c)
            CW = N // (NCHK // CT)  # 1024
            chunks = [(h, c) for h in range(NCHK // CT) for c in range(CT)]
            nc.gpsimd.dma_start(out=gam, in_=g_d[:].to_broadcast((P, 1)))

            def load(idx):
                h, c = chunks[idx]
                sl = slice(h * CW, (h + 1) * CW)
                ring = nc.sync if c % 2 == 0 else nc.scalar
                ring.dma_start(out=q[:, c, sl], in_=x_d[c * P : (c + 1) * P, sl])

            def cast(idx):
                h, c = chunks[idx]
                sl = slice(h * CW, (h + 1) * CW)
                nc.vector.tensor_copy(out=q_bf[:, c, sl], in_=q[:, c, sl])

            def transpose(idx):
                h, c = chunks[idx]
                sl = slice(h * CW, (h + 1) * CW)
                ring = nc.sync if c % 2 == 0 else nc.scalar
                ring.dma_start_transpose(out=qTs[idx], in_=q_bf[:, c, sl])

            def gather(idx):
                h, c = chunks[idx]
                ksl = slice(h * (CW // P), (h + 1) * (CW // P))
                nc.vector.tensor_copy(out=qT[:, ksl, c, :], in_=qTs[idx])

            # Interleaved pipeline on the two HWDGE rings (gpsimd/SWDGE is too
            # slow — its in-flight DMAs hold shared lane sems that stall
            # everyone). Each transpose rides 2 load-slots behind its own load
            # (cast just done when the ring reaches it); gathers sit early in
            # DVE order (Tile encodes cross-engine deps as DVE-op-count).
            # h0 (chunks 0-3) is transposed on the then-idle PE instead of the
            # xbar: its 4 chunks gate the first energy matmuls, and early xbar
            # transposes crawl while the load stream owns the SBUF ports.
            def pe_transpose(idx):
                h, c = chunks[idx]
                kb = CW // P  # k-blocks per chunk
                tp = opsum.tile([P, 4 * FD], bf16, name="tpb", tag="ops")
                for kk in range(kb):
                    nc.tensor.transpose(
                        tp[:, kk * P : (kk + 1) * P],
                        q_bf[:, c, (h * kb + kk) * P : (h * kb + kk + 1) * P],
                        ident,
                    )
                nc.vector.tensor_copy(
                    out=qT[:, h * kb : (h + 1) * kb, c, :],
                    in_=tp[:, 0:CW].rearrange("p (k v) -> p k v", v=P),
                )

            # ---- energy accumulators ----
            Es = [
                epsum.tile([P, C], fp32, name=f"E{i}", tag=f"E{i}", bufs=1)
                for i in range(CT)
            ]

            # E is symmetric: compute only column blocks j >= i; the j < i
            # blocks are mirrored from E[j] after accumulation completes.
            def energy(k0, k1, stop=False):
                for k in range(k0, k1):
                    for i in range(CT):
                        nc.tensor.matmul(
                            Es[i][:, i * P :],
                            lhsT=qT[:, k, i, :],
                            rhs=qT[:, k, i:, :],
                            start=(k == 0),
                            stop=(stop and k == KT - 1),
                        )

            for idx in range(NCHK):
                load(idx)
                for g in range(3):  # h0..h2 on PE, energy group right after
                    if idx == 4 * g + 3:
                        for j in range(4 * g, 4 * g + 4):
                            cast(j)
                        for jj in range(4 * g, 4 * g + 4):
                            pe_transpose(jj)
                        energy(8 * g, 8 * g + 8)
                if idx >= 12:
                    cast(idx)
            for idx in range(NCHK - 4, NCHK):
                transpose(idx)
            for idx in range(NCHK - 4, NCHK):
                gather(idx)

            # switch to i-outer for the tail so E[i] completes (and
            # softmax(i) starts) staggered in i; then mirror E[i, j<i] from
            # the finished E[j] row-blocks (copy out to SBUF, PE-transpose
            # back into the bank).
            for i in range(CT):
                for k in range(24, KT):
                    nc.tensor.matmul(
                        Es[i][:, i * P :],
                        lhsT=qT[:, k, i, :],
                        rhs=qT[:, k, i:, :],
                        start=False,
                        stop=(k == KT - 1),
                    )
                for j in range(i):
                    etmp = stats.tile([P, P], fp32, name="etmp", tag="etmp")
                    nc.vector.tensor_copy(
                        out=etmp, in_=Es[j][:, i * P : (i + 1) * P]
                    )
                    nc.tensor.transpose(
                        Es[i][:, j * P : (j + 1) * P], etmp, ident32
                    )

            for i in range(CT):
                E = Es[i]
                # ---- softmax (unnormalized): att = exp(mn - E), s = rowsum ----
                mn = stats.tile([P, 1], fp32)
                nc.vector.tensor_reduce(out=mn, in_=E, axis=AX, op=ALU.min)
                s = stats.tile([P, 1], fp32)
                nc.scalar.activation(
                    out=att[:, i, :],
                    in_=E,
                    func=ACT_EXP,
                    bias=mn,
                    scale=-1.0,
                    accum_out=s,
                )
                rg = stats.tile([P, 1], fp32)
                nc.vector.reciprocal(out=rg, in_=s)
                nc.vector.tensor_mul(rg, rg, gam)

                # ---- attT slab i via xbar transpose ----
                tr = nc.sync if i % 2 == 0 else nc.scalar
                tr.dma_start_transpose(out=attT[:, i, :, :], in_=att[:, i, :])

                # ---- out row-block i ----
                ot = outp.tile([P, N], fp32, name="ot", tag="ot", bufs=2)
                for nh in range(4):  # 1024-wide chunks, 2 PSUM banks each
                    sl = slice(nh * 2 * FD, (nh + 1) * 2 * FD)
                    ops = opsum.tile([P, 2 * FD], fp32, name="ops", tag="ops")
                    for half in range(2):
                        hsl = slice((nh * 2 + half) * FD, (nh * 2 + half + 1) * FD)
                        for j in range(CT):
                            nc.tensor.matmul(
                                ops[:, half * FD : (half + 1) * FD],
                                lhsT=attT[:, i, j, :],
                                rhs=q_bf[:, j, hsl],
                                start=(j == 0),
                                stop=(j == CT - 1),
                            )
                    # out = (psum * gamma/s) + x, exact fp32 add of x
                    nc.vector.scalar_tensor_tensor(
                        out=ot[:, sl],
                        in0=ops,
                        scalar=rg,
                        in1=q[:, i, sl],
                        op0=ALU.mult,
                        op1=ALU.add,
                    )
                    st = [nc.sync, nc.scalar, nc.gpsimd][(i * 4 + nh) % 3]
                    st.dma_start(out=o_d[i * P : (i + 1) * P, sl], in_=ot[:, sl])

    nc.compile()
    return nc


def _get_nc():
    if "nc" not in _CACHE:
        _CACHE["nc"] = _build_bass()
    return _CACHE["nc"]


def run(x, gamma, **run_kwargs):
    """Run on 8 cores; returns (results_list, BassKernelResults)."""
    from concourse.bass_utils import run_bass_kernel_spmd

    nc = _get_nc()
    x = np.ascontiguousarray(x, dtype=np.float32)
    gamma = np.ascontiguousarray(gamma, dtype=np.float32)
    in_maps = [
        {"x": np.ascontiguousarray(x[b].reshape(C, N)), "gamma": gamma}
        for b in range(B)
    ]
    res = run_bass_kernel_spmd(nc, in_maps, core_ids=list(range(B)), **run_kwargs)
    out = np.stack([r["out"] for r in res.results]).reshape(B, C, H, W)
    return out, res


def kernel(x, gamma):
    out, _ = run(x, gamma)
    return out.astype(np.float32)
